# revision 14
# baseline (speedup 1.0000x reference)
"""Trainium2 Bass kernel for nn_CrossAttentionFusion (B=16384, D=2048, fp32).

Math: in the reference, softmax is taken over a length-1 axis, so it is
identically 1.0 and the q/k projections are dead code:

    out = (emb_b @ Wv.T + bv + emb_a) @ Wo.T + bo
        = emb_b @ (Wv.T @ Wo.T) + emb_a @ Wo.T + (Wo @ bv + bo)

The kernel computes the fused two-matmul form.  Host precomputes
Wc = Wv.T @ Wo.T (fp32) and bias_c = Wo @ bv + bo; both output-feature
contributions accumulate in PSUM on-chip.

Sharding: data-parallel over the batch dim, 2048 rows per NeuronCore.
Layout is feature-major on-device (features on partitions, rows on the
free dim), so no transposes are ever needed on-device; the host
transposes the embedding shards in and the output shards back out.

Numerics: matmul operands are cast to bf16 on host (PE array runs bf16
at 1 cycle/row vs 4 for fp32); accumulation is fp32 in PSUM.
"""

import numpy as np
import ml_dtypes

import concourse.bass as bass
import concourse.mybir as mybir
import concourse.tile as tile
from concourse import bacc
from concourse.bass import ts
from concourse.bass_utils import run_bass_kernel_spmd

BF16 = ml_dtypes.bfloat16

NCORES = 8
B = 16384
D = 2048
R = B // NCORES          # rows per core
P = 128                  # partitions
KO = D // P              # contraction chunks (16)
MO = D // P              # output-feature chunks (16)
NT = 512                 # rows per matmul (moving free dim)
NB = R // NT             # row blocks per core (4)

_NC_CACHE = {}

# Exposed for test harnesses: BassKernelResults of the most recent run.
LAST_RESULT = None


def _build_bass(D=D, R=R, NT=NT):
    """Per-core program: outt[D, R] = Wc.T-path(ebt) + WoT-path(eat) + bias."""
    KO = D // P
    MO = D // P
    NB = R // NT
    # Bacc (not raw Bass): its compile() splits multi-sem waits into
    # InstEventSemaphore (TRN2 allows at most one sync wait per instruction).
    nc = bacc.Bacc(None, target_bir_lowering=False)
    f32 = mybir.dt.float32
    bf16 = mybir.dt.bfloat16

    NB_ = R // NT
    # Acts arrive host-pre-blocked as [NB, P, KO, NT]: one DMA per block with
    # 16KB contiguous per partition (the DMA engines are descriptor-rate
    # limited, so long lines are everything).
    ebt_d = nc.dram_tensor("ebt", [NB_, P, KO, NT], bf16, kind="ExternalInput")
    eat_d = nc.dram_tensor("eat", [NB_, P, KO, NT], bf16, kind="ExternalInput")
    wc_d = nc.dram_tensor("wc", [D, D], bf16, kind="ExternalInput")
    wot_d = nc.dram_tensor("wot", [D, D], bf16, kind="ExternalInput")
    bias_d = nc.dram_tensor("bias", [D], f32, kind="ExternalInput")
    outt_d = nc.dram_tensor("outt", [D, R], f32, kind="ExternalOutput")

    wc_r = wc_d.rearrange("(ko p) m -> p ko m", p=P)
    wot_r = wot_d.rearrange("(ko p) m -> p ko m", p=P)
    bias_r = bias_d.rearrange("(mo p) -> p mo", p=P)

    with tile.TileContext(nc) as tc:
        with (
            tc.tile_pool(name="weights", bufs=1) as wpool,
            tc.tile_pool(name="acts", bufs=2) as apool,
            tc.tile_pool(name="outs", bufs=4) as opool,
            tc.tile_pool(name="psum", bufs=8, space="PSUM") as pspool,
        ):
            wc_sb = wpool.tile([P, KO, D], bf16, tag="wc")
            wot_sb = wpool.tile([P, KO, D], bf16, tag="wot")
            bias_st = wpool.tile([P, MO], f32, tag="bias_st")
            bias_sb = wpool.tile([P, MO], f32, tag="bias")

            # PE warm-up: dummy matmuls on memset tiles while the first DMAs
            # land. ~3.4us of sustained PE activity flips the HAM clock gate
            # to 2.4GHz before real work arrives. Shares the "ps" psum tag so
            # no extra PSUM bank is needed; result is never read.
            wu_w = wpool.tile([P, P], bf16, tag="wu_w")
            wu_x = wpool.tile([P, NT], bf16, tag="wu_x")
            nc.vector.memset(wu_w[:], 0.0)
            nc.vector.memset(wu_x[:], 0.0)
            wu_ps = pspool.tile([P, NT], f32, tag="ps")
            for i in range(16):
                nc.tensor.matmul(
                    wu_ps[:], wu_w[:], wu_x[:], start=(i == 0), stop=(i == 15)
                )

            # Stage bias through a DVE copy: the per-tile bias-add TensorTensor
            # then depends only on PE (TT has a single HW sync-wait slot).
            nc.sync.dma_start(bias_st[:], bias_r[:])
            nc.vector.tensor_copy(bias_sb[:], bias_st[:])

            row_blocks = [(i * NT, NT) for i in range(NB_)]

            # Activations ride the ACT HWDGE ring (nc.scalar) so they never
            # queue behind the weight stream on the SP ring (nc.sync); one DMA
            # per block (16KB lines, 128 descriptors).
            def load_acts(nb):
                eb_t = apool.tile([P, KO, NT], bf16, tag="eb")
                ea_t = apool.tile([P, KO, NT], bf16, tag="ea")
                nc.scalar.dma_start(eb_t[:], ebt_d[nb])
                nc.scalar.dma_start(ea_t[:], eat_d[nb])
                return eb_t, ea_t

            # Weights: per-ko full-width DMAs (4KB lines), wc/wot interleaved
            # so both halves of the first psum group unblock early.
            acts_pre = load_acts(0)
            for ko in range(KO):
                nc.sync.dma_start(wc_sb[:, ko, :], wc_r[:, ko, :])
                nc.sync.dma_start(wot_sb[:, ko, :], wot_r[:, ko, :])

            for nb, (r0, w) in enumerate(row_blocks):
                eb_t, ea_t = acts_pre if nb == 0 else load_acts(nb)

                for mo in range(MO):
                    ps = pspool.tile([P, NT], f32, tag="ps")
                    for ko in range(KO):
                        nc.tensor.matmul(
                            ps[:, :w],
                            wc_sb[:, ko, ts(mo, P)],
                            eb_t[:, ko, :w],
                            start=(ko == 0),
                            stop=False,
                        )
                    for ko in range(KO):
                        nc.tensor.matmul(
                            ps[:, :w],
                            wot_sb[:, ko, ts(mo, P)],
                            ea_t[:, ko, :w],
                            start=False,
                            stop=(ko == KO - 1),
                        )
                    ot = opool.tile([P, NT], f32, tag="ot")
                    nc.vector.tensor_tensor(
                        ot[:, :w],
                        ps[:, :w],
                        bias_sb[:, mo : mo + 1].to_broadcast((P, w)),
                        mybir.AluOpType.add,
                    )
                    nc.sync.dma_start(outt_d[ts(mo, P), r0 : r0 + w], ot[:, :w])

    nc.compile()
    return nc


def kernel(emb_a, emb_b, Wq, bq, Wk, bk, Wv, bv, Wo, bo):
    global LAST_RESULT
    emb_a = np.asarray(emb_a, dtype=np.float32)
    emb_b = np.asarray(emb_b, dtype=np.float32)
    Wv = np.asarray(Wv, dtype=np.float32)
    bv = np.asarray(bv, dtype=np.float32)
    Wo = np.asarray(Wo, dtype=np.float32)
    bo = np.asarray(bo, dtype=np.float32)

    # Fused weights / bias (q/k are dead code: softmax over a length-1
    # axis is exactly 1.0).
    Wc = np.matmul(Wv.T, Wo.T)                       # [D_in, D_out] fp32
    bias = (Wo.astype(np.float64) @ bv.astype(np.float64) + bo).astype(np.float32)

    wc_bf = Wc.astype(BF16)
    wot_bf = Wo.T.astype(BF16, order="C")

    ea_bf = emb_a.astype(BF16)
    eb_bf = emb_b.astype(BF16)

    def block_acts(a):
        # [R, D] -> [NB, P, KO, NT]: element (nb, p, ko, r) = a[nb*NT+r, ko*P+p]
        nb_ = R // NT
        return np.ascontiguousarray(
            a.reshape(nb_, NT, D // P, P).transpose(0, 3, 2, 1)
        )

    in_maps = []
    for c in range(NCORES):
        sl = slice(c * R, (c + 1) * R)
        in_maps.append(
            {
                "ebt": block_acts(eb_bf[sl]),
                "eat": block_acts(ea_bf[sl]),
                "wc": wc_bf,
                "wot": wot_bf,
                "bias": bias,
            }
        )

    if "nc" not in _NC_CACHE:
        _NC_CACHE["nc"] = _build_bass()
    nc = _NC_CACHE["nc"]

    res = run_bass_kernel_spmd(nc, in_maps, core_ids=list(range(NCORES)))
    LAST_RESULT = res

    out = np.empty((B, D), dtype=np.float32)
    for c in range(NCORES):
        out[c * R : (c + 1) * R, :] = res.results[c]["outt"].T
    return out


# revision 17
# speedup vs baseline: 1.0337x; 1.0337x over previous
"""Trainium2 Bass kernel for nn_CrossAttentionFusion (B=16384, D=2048, fp32).

Math: in the reference, softmax is taken over a length-1 axis, so it is
identically 1.0 and the q/k projections are dead code:

    out = (emb_b @ Wv.T + bv + emb_a) @ Wo.T + bo
        = emb_b @ (Wv.T @ Wo.T) + emb_a @ Wo.T + (Wo @ bv + bo)

The kernel computes the fused two-matmul form.  Host precomputes
Wc = Wv.T @ Wo.T (fp32) and bias_c = Wo @ bv + bo; both output-feature
contributions accumulate in PSUM on-chip.

Sharding: data-parallel over the batch dim, 2048 rows per NeuronCore.
Layout is feature-major on-device (features on partitions, rows on the
free dim), so no transposes are ever needed on-device; the host
transposes the embedding shards in and the output shards back out.

Numerics: matmul operands are cast to bf16 on host (PE array runs bf16
at 1 cycle/row vs 4 for fp32); accumulation is fp32 in PSUM.
"""

import numpy as np
import ml_dtypes

import concourse.bass as bass
import concourse.mybir as mybir
import concourse.tile as tile
from concourse import bacc
from concourse.bass import ts
from concourse.bass_utils import run_bass_kernel_spmd

BF16 = ml_dtypes.bfloat16

NCORES = 8
B = 16384
D = 2048
R = B // NCORES          # rows per core
P = 128                  # partitions
KO = D // P              # contraction chunks (16)
MO = D // P              # output-feature chunks (16)
NT = 512                 # rows per matmul (moving free dim)
NB = R // NT             # row blocks per core (4)

_NC_CACHE = {}

# Exposed for test harnesses: BassKernelResults of the most recent run.
LAST_RESULT = None


def _build_bass(D=D, R=R, NT=NT):
    """Per-core program: outt[D, R] = Wc.T-path(ebt) + WoT-path(eat) + bias."""
    KO = D // P
    MO = D // P
    NB = R // NT
    # Bacc (not raw Bass): its compile() splits multi-sem waits into
    # InstEventSemaphore (TRN2 allows at most one sync wait per instruction).
    nc = bacc.Bacc(None, target_bir_lowering=False)
    f32 = mybir.dt.float32
    bf16 = mybir.dt.bfloat16

    NB_ = R // NT
    MBW = 2 * P                  # weight column-block width
    MB_ = D // MBW
    # Acts and weights arrive host-pre-blocked so every DMA is one long
    # contiguous run per partition (16KB / 8KB lines) — the DMA engines are
    # descriptor-rate limited, so long lines are everything. Weight blocks are
    # 1MB each so the first matmul group gates on ~6MB, not the full 16MB.
    ebt_d = nc.dram_tensor("ebt", [NB_, P, KO, NT], bf16, kind="ExternalInput")
    eat_d = nc.dram_tensor("eat", [NB_, P, KO, NT], bf16, kind="ExternalInput")
    wc_d = nc.dram_tensor("wc", [MB_, P, KO, MBW], bf16, kind="ExternalInput")
    wot_d = nc.dram_tensor("wot", [MB_, P, KO, MBW], bf16, kind="ExternalInput")
    bias_d = nc.dram_tensor("bias", [D], f32, kind="ExternalInput")
    outt_d = nc.dram_tensor("outt", [D, R], f32, kind="ExternalOutput")

    bias_r = bias_d.rearrange("(mo p) -> p mo", p=P)

    with tile.TileContext(nc) as tc:
        with (
            tc.tile_pool(name="weights", bufs=1) as wpool,
            tc.tile_pool(name="acts", bufs=2) as apool,
            tc.tile_pool(name="outs", bufs=4) as opool,
            tc.tile_pool(name="psum", bufs=8, space="PSUM") as pspool,
        ):
            wc_sb = wpool.tile([P, KO, D], bf16, tag="wc")
            wot_sb = wpool.tile([P, KO, D], bf16, tag="wot")
            bias_st = wpool.tile([P, MO], f32, tag="bias_st")
            bias_sb = wpool.tile([P, MO], f32, tag="bias")

            # PE warm-up: dummy matmuls on memset tiles while the first DMAs
            # land. ~3.4us of sustained PE activity flips the HAM clock gate
            # to 2.4GHz before real work arrives. Shares the "ps" psum tag so
            # no extra PSUM bank is needed; result is never read.
            wu_w = wpool.tile([P, P], bf16, tag="wu_w")
            wu_x = wpool.tile([P, NT], bf16, tag="wu_x")
            nc.vector.memset(wu_w[:], 0.0)
            nc.vector.memset(wu_x[:], 0.0)
            wu_ps = pspool.tile([P, NT], f32, tag="ps")
            for i in range(16):
                nc.tensor.matmul(
                    wu_ps[:], wu_w[:], wu_x[:], start=(i == 0), stop=(i == 15)
                )

            # Stage bias through a DVE copy: the per-tile bias-add TensorTensor
            # then depends only on PE (TT has a single HW sync-wait slot).
            nc.sync.dma_start(bias_st[:], bias_r[:])
            nc.vector.tensor_copy(bias_sb[:], bias_st[:])

            row_blocks = [(i * NT, NT) for i in range(NB_)]

            # Activations ride the ACT HWDGE ring (nc.scalar) so they never
            # queue behind the weight stream on the SP ring (nc.sync); one DMA
            # per block (16KB lines, 128 descriptors).
            def load_acts(nb):
                eb_t = apool.tile([P, KO, NT], bf16, tag="eb")
                ea_t = apool.tile([P, KO, NT], bf16, tag="ea")
                nc.scalar.dma_start(eb_t[:], ebt_d[nb])
                nc.scalar.dma_start(ea_t[:], eat_d[nb])
                return eb_t, ea_t

            # Weights: per-column-block DMAs (8KB lines, 1MB each), wc/wot
            # interleaved so both halves of each psum group unblock together.
            def load_w_block(mb):
                sl = ts(mb, MBW)
                nc.sync.dma_start(wc_sb[:, :, sl], wc_d[mb])
                nc.sync.dma_start(wot_sb[:, :, sl], wot_d[mb])

            load_w_block(0)
            acts_pre = load_acts(0)
            for mb in range(1, MB_):
                load_w_block(mb)

            for nb, (r0, w) in enumerate(row_blocks):
                eb_t, ea_t = acts_pre if nb == 0 else load_acts(nb)

                for mo in range(MO):
                    ps = pspool.tile([P, NT], f32, tag="ps")
                    for ko in range(KO):
                        nc.tensor.matmul(
                            ps[:, :w],
                            wc_sb[:, ko, ts(mo, P)],
                            eb_t[:, ko, :w],
                            start=(ko == 0),
                            stop=False,
                        )
                    for ko in range(KO):
                        nc.tensor.matmul(
                            ps[:, :w],
                            wot_sb[:, ko, ts(mo, P)],
                            ea_t[:, ko, :w],
                            start=False,
                            stop=(ko == KO - 1),
                        )
                    ot = opool.tile([P, NT], f32, tag="ot")
                    nc.vector.tensor_tensor(
                        ot[:, :w],
                        ps[:, :w],
                        bias_sb[:, mo : mo + 1].to_broadcast((P, w)),
                        mybir.AluOpType.add,
                    )
                    nc.sync.dma_start(outt_d[ts(mo, P), r0 : r0 + w], ot[:, :w])

    nc.compile()
    return nc


def kernel(emb_a, emb_b, Wq, bq, Wk, bk, Wv, bv, Wo, bo):
    global LAST_RESULT
    emb_a = np.asarray(emb_a, dtype=np.float32)
    emb_b = np.asarray(emb_b, dtype=np.float32)
    Wv = np.asarray(Wv, dtype=np.float32)
    bv = np.asarray(bv, dtype=np.float32)
    Wo = np.asarray(Wo, dtype=np.float32)
    bo = np.asarray(bo, dtype=np.float32)

    # Fused weights / bias (q/k are dead code: softmax over a length-1
    # axis is exactly 1.0).
    Wc = np.matmul(Wv.T, Wo.T)                       # [D_in, D_out] fp32
    bias = (Wo.astype(np.float64) @ bv.astype(np.float64) + bo).astype(np.float32)

    def block_weights(w_bf):
        # [D, D] (k, m) -> [MB, P, KO, MBW]: (mb, p, ko, m) = w[ko*P+p, mb*MBW+m]
        MBW = 2 * 128
        return np.ascontiguousarray(
            w_bf.reshape(D // 128, 128, D // MBW, MBW).transpose(2, 1, 0, 3)
        )

    wc_bf = block_weights(Wc.astype(BF16))
    wot_bf = block_weights(Wo.T.astype(BF16, order="C"))

    ea_bf = emb_a.astype(BF16)
    eb_bf = emb_b.astype(BF16)

    def block_acts(a):
        # [R, D] -> [NB, P, KO, NT]: element (nb, p, ko, r) = a[nb*NT+r, ko*P+p]
        nb_ = R // NT
        return np.ascontiguousarray(
            a.reshape(nb_, NT, D // P, P).transpose(0, 3, 2, 1)
        )

    in_maps = []
    for c in range(NCORES):
        sl = slice(c * R, (c + 1) * R)
        in_maps.append(
            {
                "ebt": block_acts(eb_bf[sl]),
                "eat": block_acts(ea_bf[sl]),
                "wc": wc_bf,
                "wot": wot_bf,
                "bias": bias,
            }
        )

    if "nc" not in _NC_CACHE:
        _NC_CACHE["nc"] = _build_bass()
    nc = _NC_CACHE["nc"]

    res = run_bass_kernel_spmd(nc, in_maps, core_ids=list(range(NCORES)))
    LAST_RESULT = res

    out = np.empty((B, D), dtype=np.float32)
    for c in range(NCORES):
        out[c * R : (c + 1) * R, :] = res.results[c]["outt"].T
    return out
